# revision 28
# baseline (speedup 1.0000x reference)
"""3-layer GraphSAGE (max-pool aggregator) on 8 trn2 NeuronCores.

Distribution (hardcoded for N=50000, E=800000, D=128, H=256, P=512, O=64):
  - Nodes are relabeled host-side: each node is assigned to a table half
    (low/high) with a balanced degree mix, then ordered within its half by
    (kA, kB) = (#low-half neighbors, #high-half neighbors) in snake order
    with outlier extraction, and dealt in 128-blocks round-robin to the 8
    cores. This makes every core's i-th node tile have near-identical
    neighbor counts, so the static per-tile gather padding is tight (~22%)
    and per-core load is balanced.
  - Each core owns 49 real tiles of 128 dst nodes. Per layer:
      h-shard = x_local @ aggW (fp16 PE, node-major rows) -> DRAM shard
      2x AllGather (low/high halves) -> full h table; each half has <32768
        rows so the custom gather's int16 indices can address it
      dma_gather (SWDGE transpose gather, single_packet=False) pulls each
        tile's padded neighbor rows feature-major with node-major index
        order; one DVE reduce_max per chunk collapses the K neighbor slices
      PE computes concat([x, hN]) @ linW (+ReLU) from the feature-major
        tiles, batched 4 node-tiles per matmul (N=512); the last layer is
        produced transposed [64 x nodes] and un-transposed on the host.
  - Sentinel table rows (-60000 / 0.0) implement segment-max padding and
    DGL's zero-fill for isolated nodes exactly; fp16 keeps relative error
    ~6e-4 (max-pooling selects exact fp16 values; matmuls accumulate f32).
"""

import sys

if "/opt/trn_rl_repo" not in sys.path:
    sys.path.insert(0, "/opt/trn_rl_repo")

import numpy as np

# ---- problem dims (hardcoded per spec) ----
N_NODES = 50000
DIM_D = 128
DIM_H = 256
DIM_P = 512
DIM_O = 64

NCORES = 8
NT = 128          # nodes per tile
CH = 16           # gather chunk (k-slices per dma_gather call)
NBIG = -60000.0   # -inf stand-in (fp16-safe)


def _geom(n_nodes):
    rt = -(-n_nodes // (NT * NCORES))          # real tiles per core
    pt = rt + 3
    if pt % 2:
        pt += 1                                # physical tiles per core (even)
    lt = pt // 2                               # tiles per half
    low_real = lt - 1                          # real tiles in low half
    high_real = rt - low_real
    assert 0 < high_real <= lt - 2, (rt, pt, lt)
    shard = pt * NT
    half = shard // 2
    assert NCORES * half < 32768, "half-table must be int16-indexable"
    return rt, pt, lt, shard, half


def _prep_graph(src, dst, n_nodes):
    """Host-side: relabel nodes, build per-core padded gather indices."""
    rt, pt, lt, shard, half = _geom(n_nodes)
    src = np.asarray(src).astype(np.int64)
    dst = np.asarray(dst).astype(np.int64)

    deg = np.bincount(dst, minlength=n_nodes)

    # --- pass 1: assign each node to a table half (low/high) ---
    # Low half holds lt-1 real tiles/core, high half rt-(lt-1).
    n_low = NCORES * (lt - 1) * NT
    n_high_real = n_nodes - n_low
    assert 0 < n_high_real <= NCORES * (lt - 2) * NT
    order0 = np.argsort(-deg, kind="stable")       # degree-desc
    # deal alternately so both halves get similar degree mix
    is_low_rank = np.zeros(n_nodes, bool)
    is_low_rank[0::2] = True
    # fix counts: currently ceil(n/2) low; move surplus/deficit from the tail
    cur_low = int(is_low_rank.sum())
    if cur_low > n_low:
        flip = np.flatnonzero(is_low_rank)[::-1][: cur_low - n_low]
        is_low_rank[flip] = False
    elif cur_low < n_low:
        flip = np.flatnonzero(~is_low_rank)[::-1][: n_low - cur_low]
        is_low_rank[flip] = True
    node_is_low = np.zeros(n_nodes, bool)
    node_is_low[order0] = is_low_rank

    # --- per-node (kA, kB): neighbor counts by src half (half-invariant
    # under within-half reordering) ---
    srcA = node_is_low[src]
    kA_n = np.bincount(dst[srcA], minlength=n_nodes)
    kB_n = deg - kA_n

    # --- pass 2: within each half, order by (kA, kB); deal 128-blocks
    # round-robin to cores so per-site K is tight across cores ---
    newid = np.empty(n_nodes, np.int64)
    for half_sel, base_tile in ((True, 0), (False, lt)):
        nodes_h = np.flatnonzero(node_is_low == half_sel)
        ka = kA_n[nodes_h].astype(np.int64)
        kb = kB_n[nodes_h].astype(np.int64)
        qa = np.quantile(ka, 0.98)
        qb = np.quantile(kb, 0.98)
        is_out = (ka > qa) | (kb > qb)
        kb_snake = np.where(ka % 2 == 0, kb, (1 << 20) - kb)
        key_bulk = np.lexsort((kb_snake, ka, is_out))
        o = nodes_h[key_bulk]
        rr = np.arange(len(o))
        blk = rr // NT
        pos = rr % NT
        core = blk % NCORES
        tl = blk // NCORES
        newid[o] = core * shard + (base_tile + tl) * NT + pos

    src_n = newid[src]
    dst_n = newid[dst]

    ec = dst_n // shard
    eloc = dst_n % shard
    ept = eloc // NT
    epos = eloc % NT
    er = np.where(ept < lt, ept, ept - 1)      # phys tile -> real tile idx
    # (real dst never sit in pad tiles; ept<lt-1 -> r=ept, ept>=lt -> r=ept-1)

    sc = src_n // shard
    sloc = src_n % shard
    isA = sloc < half
    arow = sc * half + sloc
    brow = sc * half + (sloc - half)

    cntA = np.zeros((NCORES, rt, NT), np.int32)
    cntB = np.zeros((NCORES, rt, NT), np.int32)
    np.add.at(cntA, (ec[isA], er[isA], epos[isA]), 1)
    np.add.at(cntB, (ec[~isA], er[~isA], epos[~isA]), 1)
    KA_site = np.maximum(cntA.max(axis=(0, 2)), 1)   # [rt]
    KB_site = np.maximum(cntB.max(axis=(0, 2)), 1)

    def k_within(mask):
        """k-rank of each edge within its (dst, half) group."""
        idx = np.flatnonzero(mask)
        o = idx[np.argsort(dst_n[idx], kind="stable")]
        d = dst_n[o]
        if len(d) == 0:
            return np.zeros(len(dst_n), np.int64)
        starts = np.r_[0, np.flatnonzero(np.diff(d)) + 1]
        lens = np.diff(np.r_[starts, len(d)])
        k_sorted = np.arange(len(d)) - np.repeat(starts, lens)
        karr = np.zeros(len(dst_n), np.int64)
        karr[o] = k_sorted
        return karr

    kA = k_within(isA)
    kB = k_within(~isA)

    offA = np.r_[0, np.cumsum(KA_site)] * NT   # flat idx offsets per site
    offB = np.r_[0, np.cumsum(KB_site)] * NT
    totalA = int(offA[-1])
    totalB = int(offB[-1])

    NROW_A = (lt - 1) * NT          # phys tile lt-1, pos 0 (low half pad)
    ZROW_A = NROW_A + 1
    NROW_B = (pt - 1 - lt) * NT     # phys tile pt-1, pos 0 (high half pad)
    ZROW_B = NROW_B + 1

    # node-major-within-chunk positions: chunk c covers k in [c*CH, c*CH+Kc);
    # flat pos = off + c*CH*NT + pos*Kc + (k - c*CH)
    def positions(kX, KX_site, offX, mask):
        er_m = er[mask]
        k_m = kX[mask]
        pos_m = epos[mask]
        c = k_m // CH
        Kc = np.minimum(CH, KX_site[er_m] - c * CH)
        return offX[er_m] + c * CH * NT + pos_m * Kc + (k_m - c * CH)

    flatA = np.full((NCORES, totalA), NROW_A, np.int32)
    flatA[ec[isA], positions(kA, KA_site, offA, isA)] = arow[isA]
    flatB = np.full((NCORES, totalB), NROW_B, np.int32)
    flatB[ec[~isA], positions(kB, KB_site, offB, ~isA)] = brow[~isA]

    z = np.flatnonzero(deg == 0)
    if len(z):
        zn = newid[z]
        zc = zn // shard
        zl = zn % shard
        zpt = zl // NT
        zr = np.where(zpt < lt, zpt, zpt - 1)
        zpos = zl % NT
        kc0 = np.minimum(CH, KA_site[zr])
        flatA[zc, offA[zr] + zpos * kc0] = ZROW_A

    assert flatA.max() < 32768 and flatB.max() < 32768

    def wrap(flat):
        t = flat.shape[1]
        a = flat.astype(np.int16).reshape(NCORES, t // 16, 16).transpose(0, 2, 1)
        return np.ascontiguousarray(np.tile(a, (1, 8, 1)))  # [NCORES,128,t//16]

    return dict(
        geom=(rt, pt, lt, shard, half),
        newid=newid,
        idxA=wrap(flatA),
        idxB=wrap(flatB),
        KA_site=KA_site.astype(int),
        KB_site=KB_site.astype(int),
        offA=offA.astype(int),
        offB=offB.astype(int),
    )


def _build_program(geom, KA_site, KB_site, offA, offB, totalA, totalB,
                   dim_d, dim_h, dim_p, dim_o, reps=1, do_coll=True, do_gather=True):
    import concourse.bacc as bacc
    import concourse.mybir as mybir
    import concourse.tile as tile
    from concourse.library_config import mlp

    fp16 = mybir.dt.float16
    f32 = mybir.dt.float32
    i16 = mybir.dt.int16
    Relu = mybir.ActivationFunctionType.Relu

    rt, pt, lt, shard, half = geom
    PC = dim_p // 128
    layer_dims = [(dim_d, dim_h, True), (dim_h, dim_h, True), (dim_h, dim_o, False)]

    nc = bacc.Bacc(
        "TRN2",
        num_devices=NCORES,
        debug=False,
        target_bir_lowering=False,
        dynamic_dma_scratch_size=32768,
    )

    xt0_d = nc.dram_tensor("xt0", [128, shard], fp16, kind="ExternalInput")
    idxA_d = nc.dram_tensor("idxA", [16, totalA // 16], i16, kind="ExternalInput")
    idxB_d = nc.dram_tensor("idxB", [16, totalB // 16], i16, kind="ExternalInput")
    padrows_d = nc.dram_tensor("padrows", [128, dim_p], fp16, kind="ExternalInput")
    aggw_d = []
    linw_d = []
    for li, (din, dout, _) in enumerate(layer_dims):
        kd = din // 128
        aggw_d.append(
            nc.dram_tensor(f"aggw{li}", [128, kd, dim_p], fp16, kind="ExternalInput")
        )
        linw_d.append(
            nc.dram_tensor(f"linw{li}", [128, kd + PC, dout], fp16, kind="ExternalInput")
        )
    out_d = nc.dram_tensor("out", [dim_o, shard], f32, kind="ExternalOutput")

    def phys(r):
        return r if r < lt - 1 else r + 1

    n_low_r = min(lt - 1, rt)
    low_rs = list(range(n_low_r))
    high_rs = list(range(n_low_r, rt))

    def chunkify(lst, n):
        return [lst[i:i + n] for i in range(0, len(lst), n)]

    groups = chunkify(low_rs, 4) + chunkify(high_rs, 4)
    pair_groups = chunkify(low_rs, 2) + chunkify(high_rs, 2)

    with tile.TileContext(nc) as tc:
        with (
            tc.tile_pool(name="const", bufs=1) as const,
            tc.tile_pool(name="work", bufs=3) as work,
            tc.tile_pool(name="gp", bufs=3) as gp,
            tc.tile_pool(name="ps", bufs=2, space="PSUM") as ps,
            tc.tile_pool(name="dram", bufs=1, space="DRAM") as dram,
        ):
            nc.gpsimd.load_library(mlp)

            # persistent SBUF state
            xta = const.tile([128, 2, shard], fp16, tag="xta")
            xtb = const.tile([128, 2, shard], fp16, tag="xtb")
            idxA_sb = const.tile([128, totalA // 16], i16, tag="idxA_sb")
            idxB_sb = const.tile([128, totalB // 16], i16, tag="idxB_sb")
            padrows = const.tile([128, dim_p], fp16, tag="padrows")
            nc.sync.dma_start(xta[:, 0, :], xt0_d[:])
            for gi in range(8):
                nc.sync.dma_start(idxA_sb[gi * 16:(gi + 1) * 16, :], idxA_d[:])
                nc.sync.dma_start(idxB_sb[gi * 16:(gi + 1) * 16, :], idxB_d[:])
            nc.sync.dma_start(padrows[:], padrows_d[:])
            aggw_sb = []
            linw_sb = []
            for li, (din, dout, _) in enumerate(layer_dims):
                kd = din // 128
                aw = const.tile([128, kd, dim_p], fp16, tag=f"aggw{li}",
                                name=f"aggw{li}_sb")
                nc.sync.dma_start(aw[:], aggw_d[li][:])
                lw = const.tile([128, kd + PC, dout], fp16, tag=f"linw{li}",
                                name=f"linw{li}_sb")
                nc.sync.dma_start(lw[:], linw_d[li][:])
                aggw_sb.append(aw)
                linw_sb.append(lw)

            # DRAM shards/tables, one set per (rep, layer)
            nset = 3 * reps
            shardA = [dram.tile([half, dim_p], fp16, tag=f"shardA{i}",
                                name=f"shardA{i}") for i in range(nset)]
            shardB = [dram.tile([half, dim_p], fp16, tag=f"shardB{i}",
                                name=f"shardB{i}") for i in range(nset)]
            tableA = [dram.tile([NCORES * half, dim_p], fp16, addr_space="Shared",
                                tag=f"tableA{i}", name=f"tableA{i}") for i in range(nset)]
            tableB = [dram.tile([NCORES * half, dim_p], fp16, addr_space="Shared",
                                tag=f"tableB{i}", name=f"tableB{i}") for i in range(nset)]

            for rep in range(reps):
             for li, (din, dout, act) in enumerate(layer_dims):
                par = rep * 3 + li
                kd = din // 128
                xin = xta if li % 2 == 0 else xtb
                xout = xtb if li % 2 == 0 else xta
                aggw = aggw_sb[li]
                linw = linw_sb[li]

                # ---- phase A: h shard = x_local @ aggW (tile pairs) ----
                for pr in pair_groups:
                    np_ = len(pr)
                    ph = ps.tile([128, 2, dim_p], f32, tag="ph", name="ph")
                    for j, r in enumerate(pr):
                        p_ = phys(r)
                        for t in range(kd):
                            nc.tensor.matmul(
                                ph[:, j, :],
                                xin[:, t, p_ * NT:(p_ + 1) * NT],
                                aggw[:, t, :],
                                start=(t == 0),
                                stop=(t == kd - 1),
                            )
                    h16 = work.tile([128, np_, dim_p], fp16, tag="h16", name="h16")
                    nc.scalar.copy(h16[:], ph[:, 0:np_, :])
                    p0 = phys(pr[0])
                    if p0 < lt:
                        dst_ap = shardA[par][p0 * NT:(p0 + np_) * NT, :]
                    else:
                        dst_ap = shardB[par][(p0 - lt) * NT:(p0 - lt + np_) * NT, :]
                    nc.sync.dma_start(
                        dst_ap.rearrange("(a b) c -> b a c", b=NT), h16[:]
                    )
                # pad tiles (sentinel rows)
                real_phys = {phys(r) for r in range(rt)}
                for p_ in range(pt):
                    if p_ in real_phys:
                        continue
                    if p_ < lt:
                        dst_ap = shardA[par][p_ * NT:(p_ + 1) * NT, :]
                    else:
                        dst_ap = shardB[par][(p_ - lt) * NT:(p_ - lt + 1) * NT, :]
                    nc.sync.dma_start(dst_ap, padrows[:])

                # ---- phase B: allgather halves ----
                if do_coll:
                 nc.gpsimd.collective_compute(
                    "AllGather",
                    mybir.AluOpType.bypass,
                    replica_groups=[list(range(NCORES))],
                    ins=[shardA[par][:].opt()],
                    outs=[tableA[par][:].opt()],
                )
                if do_coll:
                 nc.gpsimd.collective_compute(
                    "AllGather",
                    mybir.AluOpType.bypass,
                    replica_groups=[list(range(NCORES))],
                    ins=[shardB[par][:].opt()],
                    outs=[tableB[par][:].opt()],
                 )

                # ---- phase C/D: gather -> reduce-max -> linear (groups) ----
                for grp in groups:
                    gs = len(grp)
                    p0 = phys(grp[0])
                    hNA = work.tile([128, PC, gs * NT], fp16, tag="hNA", name="hNA")
                    hNB = work.tile([128, PC, gs * NT], fp16, tag="hNB", name="hNB")
                    for half_i, (table, off, Ks, idx_sb, hNX) in enumerate((
                        (tableA[par], offA, KA_site, idxA_sb, hNA),
                        (tableB[par], offB, KB_site, idxB_sb, hNB),
                    )):
                        for j, r in enumerate(grp):
                            K = int(Ks[r])
                            base = int(off[r])
                            out_sl = hNX[:, :, j * NT:(j + 1) * NT]
                            k0 = 0
                            first = True
                            while k0 < K:
                                kc = min(CH, K - k0)
                                nidx = kc * NT
                                g = gp.tile([128, PC, nidx], fp16, tag="g", name="g")
                                c0 = (base + k0 * NT) // 16
                                if do_gather:
                                    nc.gpsimd.dma_gather(
                                        g[:], table[:],
                                        idx_sb[:, c0:c0 + nidx // 16],
                                        nidx, nidx, dim_p,
                                        transpose=True, single_packet=False,
                                    )
                                else:
                                    nc.vector.memset(g[:], 0.0)
                                gv = g.rearrange("p c (n k) -> p c n k", k=kc)
                                if first:
                                    nc.vector.reduce_max(
                                        out_sl, gv, axis=mybir.AxisListType.X
                                    )
                                    first = False
                                else:
                                    tmp = work.tile([128, PC, NT], fp16,
                                                    tag="tmp", name="tmp")
                                    nc.vector.reduce_max(
                                        tmp[:], gv, axis=mybir.AxisListType.X
                                    )
                                    nc.vector.tensor_max(out_sl, out_sl, tmp[:])
                                k0 += kc
                    # merge halves: hNA = max(hNA, hNB) in one op
                    nc.vector.tensor_max(hNA[:], hNA[:], hNB[:])

                    if act:
                        for hc in range(dout // 128):
                            po = ps.tile([128, gs * NT], f32, tag="po", name="po")
                            for t in range(kd):
                                nc.tensor.matmul(
                                    po[:],
                                    linw[:, t, hc * 128:(hc + 1) * 128],
                                    xin[:, t, p0 * NT:(p0 + gs) * NT],
                                    start=(t == 0),
                                    stop=False,
                                )
                            for t4 in range(PC):
                                nc.tensor.matmul(
                                    po[:],
                                    linw[:, kd + t4, hc * 128:(hc + 1) * 128],
                                    hNA[:, t4, :],
                                    start=False,
                                    stop=(t4 == PC - 1),
                                )
                            nc.scalar.activation(
                                xout[:, hc, p0 * NT:(p0 + gs) * NT], po[:], Relu
                            )
                    else:
                        po = ps.tile([dim_o, gs * NT], f32, tag="po", name="po")
                        for t in range(kd):
                            nc.tensor.matmul(
                                po[:],
                                linw[:, t, 0:dim_o],
                                xin[:, t, p0 * NT:(p0 + gs) * NT],
                                start=(t == 0),
                                stop=False,
                            )
                        for t4 in range(PC):
                            nc.tensor.matmul(
                                po[:],
                                linw[:, kd + t4, 0:dim_o],
                                hNA[:, t4, :],
                                start=False,
                                stop=(t4 == PC - 1),
                            )
                        o32 = work.tile([dim_o, gs * NT], f32, tag="o32", name="o32")
                        nc.scalar.copy(o32[:], po[:])
                        nc.sync.dma_start(
                            out_d[:, p0 * NT:(p0 + gs) * NT], o32[:]
                        )

    nc.compile()
    return nc


def _weights_maps(aggWs, linWs, dim_p):
    maps = {}
    for li, (aggW, linW) in enumerate(zip(aggWs, linWs)):
        din = aggW.shape[0]
        kd = din // 128
        maps[f"aggw{li}"] = np.ascontiguousarray(
            aggW.astype(np.float16).reshape(kd, 128, dim_p).transpose(1, 0, 2)
        )
        kt = linW.shape[0] // 128
        dout = linW.shape[1]
        maps[f"linw{li}"] = np.ascontiguousarray(
            linW.astype(np.float16).reshape(kt, 128, dout).transpose(1, 0, 2)
        )
    return maps


def _run(x, src, dst, aggWs, linWs, n_nodes, dim_d, dim_h, dim_p, dim_o):
    from concourse.bass_utils import run_bass_kernel_spmd

    prep = _prep_graph(src, dst, n_nodes)
    rt, pt, lt, shard, half = prep["geom"]
    totalA = prep["idxA"].shape[2] * 16
    totalB = prep["idxB"].shape[2] * 16

    nc = _build_program(
        prep["geom"], prep["KA_site"], prep["KB_site"], prep["offA"], prep["offB"],
        totalA, totalB, dim_d, dim_h, dim_p, dim_o,
    )

    newid = prep["newid"]
    X = np.zeros((NCORES * shard, dim_d), np.float16)
    X[newid] = np.asarray(x, np.float32).astype(np.float16)
    padrows = np.full((128, dim_p), NBIG, np.float16)
    padrows[1, :] = 0.0
    wmaps = _weights_maps(aggWs, linWs, dim_p)

    in_maps = []
    for c in range(NCORES):
        m = dict(wmaps)
        m["xt0"] = np.ascontiguousarray(X[c * shard:(c + 1) * shard].T)
        m["idxA"] = np.ascontiguousarray(prep["idxA"][c][:16])
        m["idxB"] = np.ascontiguousarray(prep["idxB"][c][:16])
        m["padrows"] = padrows
        in_maps.append(m)

    res = run_bass_kernel_spmd(nc, in_maps, core_ids=list(range(NCORES)))
    big = np.concatenate([res.results[c]["out"].T for c in range(NCORES)], axis=0)
    return np.ascontiguousarray(big[newid]).astype(np.float32)


def kernel(x, src, dst, aggW0, aggW1, aggW2, linW0, linW1, linW2):
    return _run(
        np.asarray(x, np.float32),
        np.asarray(src),
        np.asarray(dst),
        [np.asarray(aggW0, np.float32), np.asarray(aggW1, np.float32),
         np.asarray(aggW2, np.float32)],
        [np.asarray(linW0, np.float32), np.asarray(linW1, np.float32),
         np.asarray(linW2, np.float32)],
        N_NODES, DIM_D, DIM_H, DIM_P, DIM_O,
    )
